# revision 58
# baseline (speedup 1.0000x reference)
"""Trainium2 Bass kernel for nn_BasicBlock (conv3x3-BN-perelem_act-conv3x3-BN + act shortcut).

Data-parallel over batch: 32 images -> 4 per core x 8 cores.

Per-core layout: each 64x112x112 image is split into top/bottom 56-row halves,
mapped to SBUF partitions 0-63 (top, one per channel) and 64-127 (bottom), so
every elementwise op runs with all 128 lanes and the per-element activation
mask arrays need only a single copy.

Conv3x3 = 9 accumulating K=128 matmuls per 8-row output chunk using
BLOCK-DIAGONAL weights diag(W_tap, W_tap) [128x128]: one full-width matmul
computes both image halves at once (the cost model charges out-free-size only,
so this halves PE time vs two 64x64 array-tile matmuls).

Per-element activation (codes 0..3 = relu/identity/tanh/sigmoid) is computed
as   act(z) = (sigmoid(s1*z) + f) * w2
with host-precomputed per-element arrays:
  s1 = {relu: 512, id: 0, tanh: 2, sigmoid: 1}
  f  = {id: +0.5, tanh: -0.5, else 0}
  w2 = z, overwritten where code in {tanh, sigmoid} with CD = {tanh: 2,
       sigmoid: 1} via one copy_predicated
(identity: (0.5+0.5)*z = z; tanh: (sig(2z)-0.5)*2; relu: step(z)*z.)
BN is folded exactly via the scalar-engine eviction z = Copy(psum*a + b)
with per-channel scale a = gamma/sqrt(var+eps) (Identity act func) and bias b = beta - mean*a.

The shortcut act reads x from the SBUF input tile (no DRAM reload); its
copy_predicated overwrites the input tile in place (conv1 is done with it).
"""

import os
import sys

sys.path.insert(0, "/opt/trn_rl_repo")

import numpy as np
from contextlib import ExitStack

import concourse.bass as bass
import concourse.bacc as bacc
import concourse.tile as tile
import concourse.mybir as mybir
from concourse.bass_utils import run_bass_kernel_spmd

F16 = np.float16
MDT = mybir.dt.float16
EPS = 1e-5
KREL = 512.0   # sigmoid(KREL*z) ~ step(z) for the relu branch

B, C, H, W = 32, 64, 112, 112
NCORES = 8
BPC = B // NCORES          # images per core
SEC = H // 2               # rows per half-section (56)
HP, WP = SEC + 2, W + 2    # padded section: 58 x 114
NU = SEC // 8              # 8-row elementwise units per half (7)

TAPS = [(ky, kx) for ky in (-1, 0, 1) for kx in (0, -1, 1)]
# out-column range per kx: the edge column whose input is zero padding is
# skipped (kx=-1 contributes nothing to out col 0; kx=+1 nothing to col 111)
TAPCOLS = {-1: (1, 112), 0: (0, 112), 1: (0, 111)}

LAST_RESULT = None  # BassKernelResults of the most recent kernel() call


def _split_halves(m):
    """[64, 112, X] -> [128, 56, X]: top rows on partitions 0-63, bottom on 64-127."""
    return np.concatenate([m[:, 0:SEC, :], m[:, SEC:H, :]], axis=0)


def _pad_split_image(img):
    """[64,112,112] fp -> [128, 58, 114] f16 padded split layout (1px halo)."""
    p = np.zeros((C, H + 2, W + 2), np.float32)
    p[:, 1:113, 1:113] = img
    top = p[:, 0:HP, :]
    bot = p[:, SEC:SEC + HP, :]
    return np.concatenate([top, bot], axis=0).astype(F16)


def _mask_arrays(codes):
    """codes [C*H*W] int32 -> dict of split-layout [128,56,112] arrays."""
    c = codes.reshape(C, H, W)
    s1 = np.select([c == 0, c == 1, c == 2, c == 3], [KREL, 0.0, 2.0, 1.0]).astype(np.float32)
    f = np.select([c == 1, c == 2], [0.5, -0.5], 0.0).astype(np.float32)
    cd = np.select([c == 2, c == 3], [2.0, 1.0], 0.0).astype(np.float32)
    return {
        "s1": _split_halves(s1).astype(F16),
        "f": _split_halves(f).astype(F16),
        "cd": _split_halves(cd).astype(F16),
        "cm": _split_halves((cd != 0).astype(np.float32)).astype(np.uint8),
    }


def _build_program(fuse_b2_zero=True):
    """fuse_b2_zero: when the BN2 additive fold b2 = beta2 - mean2*a2 is
    identically zero (true for the benchmark fills), the very last output
    group fuses eviction+residual-add into one DVE scalar_tensor_tensor,
    shortening the post-final-matmul tail."""
    nc = bacc.Bacc("TRN2", target_bir_lowering=False, debug=False)

    xin = nc.dram_tensor("xin", [BPC, 128, HP, WP], MDT, kind="ExternalInput")
    w1d = nc.dram_tensor("w1", [128, 9, 64], MDT, kind="ExternalInput")
    w2d = nc.dram_tensor("w2", [128, 9, 128], MDT, kind="ExternalInput")
    scld = nc.dram_tensor("scl", [128, 4], mybir.dt.float32, kind="ExternalInput")
    mnames = ["s1f", "ff", "cdf", "s1s", "fs", "cds"]
    mdram = {
        k: nc.dram_tensor(k, [128, SEC, W], MDT, kind="ExternalInput") for k in mnames
    }
    for k in ("cmf", "cms"):  # uint8 predicate masks
        mdram[k] = nc.dram_tensor(k, [128, SEC, W], mybir.dt.uint8, kind="ExternalInput")
    outd = nc.dram_tensor("out", [BPC, 128, SEC, W], MDT, kind="ExternalOutput")

    IDN = mybir.ActivationFunctionType.Identity
    SG = mybir.ActivationFunctionType.Sigmoid
    BYP = mybir.AluOpType.bypass
    ADD = mybir.AluOpType.add

    with tile.TileContext(nc) as tc, ExitStack() as ctx:
        wp = ctx.enter_context(tc.tile_pool(name="w", bufs=1))
        mp = ctx.enter_context(tc.tile_pool(name="m", bufs=1))
        xp = ctx.enter_context(tc.tile_pool(name="x", bufs=3))
        hp = ctx.enter_context(tc.tile_pool(name="h", bufs=2))
        ep = ctx.enter_context(tc.tile_pool(name="e", bufs=2))
        op_ = ctx.enter_context(tc.tile_pool(name="o", bufs=3))
        pp = ctx.enter_context(tc.tile_pool(name="ps", bufs=4, space="PSUM"))

        w1t = wp.tile([128, 9, 128], MDT, tag="w1")
        w1s = wp.tile([128, 9, 64], MDT, tag="w1s")
        w2t = wp.tile([128, 9, 128], MDT, tag="w2")
        sclt = wp.tile([128, 4], mybir.dt.float32, tag="scl")
        a1t, b1t, a2t, b2t = (sclt[:, i:i + 1] for i in range(4))
        mt = {}
        for k in mnames:
            mt[k] = mp.tile([128, SEC, W], MDT, tag=k, name=k)
        for k in ("cmf", "cms"):
            mt[k] = mp.tile([128, SEC, W], mybir.dt.uint8, tag=k, name=k)

        def conv_unit(src, wt, ps, r0):
            """9-tap block-diag conv into psum ps[:, 0:8, 0:112] for output
            rows r0..r0+7 of both halves at once (K=128, M=128). The first
            tap is full-width kx=0 (start=True must cover every column);
            kx=+-1 taps skip their zero-padding edge column."""
            for i in (0, 1):
                for t, (ky, kx) in enumerate(TAPS):
                    rs = r0 + 4 * i + 1 + ky
                    c0, c1 = TAPCOLS[kx]
                    nc.tensor.matmul(
                        ps[:, 4 * i:4 * i + 4, c0:c1], wt[:, t, :],
                        src[:, rs:rs + 4, c0 + kx + 1:c1 + kx + 1],
                        start=(t == 0), stop=(t == 8),
                        skip_group_check=True,
                    )

        xts = {}
        hts = {}

        def load_x(n):
            if n >= BPC:
                return
            xts[n] = xp.tile([128, HP, WP], MDT, tag="xt", name=f"xt{n}")
            nc.sync.dma_start(xts[n][:], xin[n, :, :, :])

        def phase_l1(n):
            """conv1 -> BN1 -> per-element act -> ht; prefetches xt(n+1)."""
            ht = hp.tile([128, HP, WP], MDT, tag="ht")
            hts[n] = ht
            if n < 2:
                # borders stay zero across buffer reuses; interior rows/cols
                # are fully rewritten every image (halo rows every image)
                nc.gpsimd.memset(ht[:, 0, :], 0.0)
                nc.gpsimd.memset(ht[:, HP - 1, :], 0.0)
                nc.gpsimd.memset(ht[:, :, 0], 0.0)
                nc.gpsimd.memset(ht[:, :, WP - 1], 0.0)
            xt = xts[n]
            # 1-unit emission lag for the chain tail (ht mul): keeps each
            # in-order engine queue from head-of-line blocking on the
            # cross-engine z->xs->sigmoid->(+f)->mul dependency ring
            lag = []

            def flush_l1(item):
                lag.append(item)
                if len(lag) < 2:
                    return
                sgp, zp, rp = lag.pop(0)
                sg2 = ep.tile([128, 8, 112], MDT, tag="sg2", bufs=2)
                nc.vector.tensor_add(sg2[:], sgp[:], mt["ff"][:, rp:rp + 8, :])
                nc.vector.tensor_mul(ht[:, rp + 1:rp + 9, 1:113], sg2[:], zp[:])

            for u in range(NU):
                r0 = 8 * u
                ps = pp.tile([128, 8, 128], mybir.dt.float32, tag="ps")
                conv_unit(xt, w1t, ps, r0)
                psv = ps[:, :, 0:112]
                z = ep.tile([128, 8, 112], MDT, tag="z", bufs=3)
                nc.scalar.activation(z[:], psv, IDN, scale=a1t[:], bias=b1t[:])
                xs = ep.tile([128, 8, 112], MDT, tag="xs")
                nc.vector.tensor_mul(xs[:], z[:], mt["s1f"][:, r0:r0 + 8, :])
                sg = ep.tile([128, 8, 112], MDT, tag="sg", bufs=3)
                nc.scalar.activation(sg[:], xs[:], SG)
                nc.vector.copy_predicated(
                    z[:], mt["cmf"][:, r0:r0 + 8, :], mt["cdf"][:, r0:r0 + 8, :])
                flush_l1((sg, z, r0))
            flush_l1((None, None, None))

            # halo exchange between the two halves of ht (row 56 of the image
            # is the bottom half's first output row; row 55 is the top's last)
            nc.sync.dma_start(ht[0:64, HP - 1, 1:113], ht[64:128, 1, 1:113])
            nc.sync.dma_start(ht[64:128, 0, 1:113], ht[0:64, SEC, 1:113])

        def phase_l2(n):
            """conv2 -> BN2 (+ shortcut act(x)) -> out"""
            ht = hts.pop(n)
            xt = xts.pop(n)
            lag = []

            def flush_l2(item, o_on_dve=False):
                lag.append(item)
                if len(lag) < 2:
                    return
                sgsp, z2p, rp = lag.pop(0)
                xvp = xt[:, rp + 1:rp + 9, 1:113]
                sgs2 = ep.tile([128, 8, 112], MDT, tag="sgs2", bufs=2)
                nc.vector.tensor_add(sgs2[:], sgsp[:], mt["fs"][:, rp:rp + 8, :])
                sc = ep.tile([128, 8, 112], MDT, tag="sc")
                nc.vector.tensor_mul(sc[:], sgs2[:], xvp)
                o = op_.tile([128, 8, 112], MDT, tag="o")
                if o_on_dve:
                    # the program-final drain: DVE (527ns) instead of Pool
                    # (1873ns) so this store clears SP/HWDGE well before the
                    # last unit's stores need them
                    nc.vector.tensor_add(o[:], z2p[:], sc[:])
                else:
                    nc.gpsimd.tensor_add(o[:], z2p[:], sc[:])
                nc.sync.dma_start(outd[n, :, rp:rp + 8, :], o[:])

            last = (n == BPC - 1)
            for u in range(NU):
                r0 = 8 * u
                xv = xt[:, r0 + 1:r0 + 9, 1:113]
                if last and u == NU - 1:
                    # final unit of the whole program: hoist the
                    # conv-independent shortcut chain ahead of the conv so
                    # only z2 + o + store remain after the last matmul
                    xss = ep.tile([128, 8, 112], MDT, tag="xss")
                    nc.vector.tensor_mul(xss[:], xv, mt["s1s"][:, r0:r0 + 8, :])
                    sgs = ep.tile([128, 8, 112], MDT, tag="sgs", bufs=3)
                    nc.scalar.activation(sgs[:], xss[:], SG)
                    sgs2l = ep.tile([128, 8, 112], MDT, tag="sgs2", bufs=2)
                    nc.vector.tensor_add(sgs2l[:], sgs[:], mt["fs"][:, r0:r0 + 8, :])
                    nc.vector.copy_predicated(
                        xv, mt["cms"][:, r0:r0 + 8, :], mt["cds"][:, r0:r0 + 8, :])
                    scl_ = ep.tile([128, 8, 112], MDT, tag="sc", name="sc_last")
                    nc.vector.tensor_mul(scl_[:], sgs2l[:], xv)
                if last and u == NU - 1:
                    flush_l2((None, None, None), o_on_dve=True)
                    # very last unit: each 4-row group gets its own psum ring
                    # tile (a shared tile would WAR-serialize group 1's
                    # matmuls behind group 0's eviction read) and is evicted
                    # and stored as soon as its 9-tap accumulation completes,
                    # so only a 4-row evict+store chain trails the last matmul
                    for i in (0, 1):
                        psh = pp.tile([128, 8, 128], mybir.dt.float32,
                                      tag="ps", name=f"ps_last{i}")
                        for t, (ky, kx) in enumerate(TAPS):
                            rs = r0 + 4 * i + 1 + ky
                            c0, c1 = TAPCOLS[kx]
                            nc.tensor.matmul(
                                psh[:, 0:4, c0:c1], w2t[:, t, :],
                                ht[:, rs:rs + 4, c0 + kx + 1:c1 + kx + 1],
                                start=(t == 0), stop=(t == 8),
                                skip_group_check=True,
                            )
                        oh = op_.tile([128, 4, 112], MDT, tag="oh", bufs=2)
                        if fuse_b2_zero:
                            nc.vector.scalar_tensor_tensor(
                                oh[:], psh[:, 0:4, 0:112], a2t[:],
                                scl_[:, 4 * i:4 * i + 4, :],
                                mybir.AluOpType.mult, ADD)
                        else:
                            z2h = ep.tile([128, 4, 112], MDT, tag="z2h", bufs=2)
                            nc.scalar.activation(z2h[:], psh[:, 0:4, 0:112],
                                                 IDN, scale=a2t[:], bias=b2t[:])
                            nc.vector.tensor_add(oh[:], z2h[:],
                                                 scl_[:, 4 * i:4 * i + 4, :])
                        nc.sync.dma_start(
                            outd[n, :, r0 + 4 * i:r0 + 4 * i + 4, :], oh[:])
                    break
                ps = pp.tile([128, 8, 128], mybir.dt.float32, tag="ps")
                conv_unit(ht, w2t, ps, r0)
                psv = ps[:, :, 0:112]
                z2 = ep.tile([128, 8, 112], MDT, tag="z2", bufs=3)
                nc.scalar.activation(z2[:], psv, IDN, scale=a2t[:], bias=b2t[:])
                xss = ep.tile([128, 8, 112], MDT, tag="xss")
                nc.vector.tensor_mul(xss[:], xv, mt["s1s"][:, r0:r0 + 8, :])
                sgs = ep.tile([128, 8, 112], MDT, tag="sgs", bufs=3)
                nc.scalar.activation(sgs[:], xss[:], SG)
                nc.vector.copy_predicated(
                    xv, mt["cms"][:, r0:r0 + 8, :], mt["cds"][:, r0:r0 + 8, :])
                flush_l2((sgs, z2, r0))
                if n == 1 and u == 1:
                    xts[3] = xp.tile([128, HP, WP], MDT, tag="xt", name="xt3")
                    nc.gpsimd.dma_start(xts[3][:], xin[3, :, :, :])
            if not last:
                flush_l2((None, None, None))

        # startup DMA order matters: the DMA-engines device is exclusive, so
        # issue what the first conv needs (w1, xt0 leading rows) before the
        # bulk mask load. xt(0) is row-chunked so conv unit 0 starts early.
        # first rows of xt0 go on the scalar queue (parallel to SP) so the
        # first conv unit's data and w1 transfer concurrently
        xt0 = xp.tile([128, HP, WP], MDT, tag="xt", name="xt0")
        xts[0] = xt0
        nc.scalar.dma_start(xt0[:, 0:6, :], xin[0, :, 0:6, :])
        # w1 loads compact (contiguous 1152B runs: no small-descriptor DMA
        # penalty) and is expanded to block-diagonal on the DVE; the padded
        # direct load would pay 2x for its 128B runs and gate the first conv
        nc.vector.memset(w1t[:], 0.0)
        nc.sync.dma_start(w1s[:], w1d[:, :, :])
        nc.scalar.dma_start(xt0[:, 6:10, :], xin[0, :, 6:10, :])
        nc.sync.dma_start(sclt[:], scld[:, :])
        nc.vector.tensor_copy(w1t[0:64, :, 0:64], w1s[0:64, :, :])
        nc.vector.tensor_copy(w1t[64:128, :, 64:128], w1s[64:128, :, :])
        # L1 masks (small leading chunk so unit-0 elementwise starts early)
        # interleaved with the rest of xt0; conv2 weights + shortcut masks last
        nc.sync.dma_start(xt0[:, 10:26, :], xin[0, :, 10:26, :])
        for k in ("s1f", "ff", "cdf", "cmf"):
            nc.sync.dma_start(mt[k][:, 0:8, :], mdram[k][:, 0:8, :])
        nc.sync.dma_start(xt0[:, 26:42, :], xin[0, :, 26:42, :])
        for k in ("s1f", "ff", "cdf", "cmf"):
            nc.sync.dma_start(mt[k][:, 8:32, :], mdram[k][:, 8:32, :])
        nc.sync.dma_start(xt0[:, 42:HP, :], xin[0, :, 42:HP, :])
        xt1 = xp.tile([128, HP, WP], MDT, tag="xt", name="xt1")
        xts[1] = xt1
        nc.sync.dma_start(xt1[:], xin[1, :, :, :])
        for k in ("s1f", "ff", "cdf", "cmf"):
            nc.sync.dma_start(mt[k][:, 32:SEC, :], mdram[k][:, 32:SEC, :])
        nc.sync.dma_start(w2t[:], w2d[:, :, :])
        for r0, r1 in [(0, 28), (28, SEC)]:
            for k in ("s1s", "fs", "cds", "cms"):
                nc.sync.dma_start(mt[k][:, r0:r1, :], mdram[k][:, r0:r1, :])
        # xt2 preloads from the same ordered SP queue (needed only at ~74us;
        # issuing it here keeps the exclusive DMA device's service order
        # deterministic). xt3 reuses xt0's buffer, so it is prefetched from
        # the Pool queue mid-L2(1), after its WAR dependency has cleared.
        load_x(2)

        # zero scratch column for ht border initialization on DVE (a strided
        # memset would otherwise lower to the Pool engine and keep it in the
        # program's barrier set)
        zcol = wp.tile([128, HP], MDT, tag="zcol")
        nc.vector.memset(zcol[:], 0.0)

        # PE p-state warmup: ~3us of scratch matmuls while the first input
        # chunks transfer, so real matmuls start at the full 2.4 GHz rate.
        # Results land in the first psum ring buffer and are never read
        # (the later write-after-write reuse is tracked by the tile pool).
        wu = wp.tile([128, 448], MDT, tag="wu")
        nc.vector.memset(wu[:], 0.0)
        wups = pp.tile([128, 8, 128], mybir.dt.float32, tag="ps", name="warm")
        for i in range(8):
            nc.tensor.matmul(wups[:, 0:4, 0:112], wu[:, 0:128], wu[:, 0:448],
                             start=True, stop=True, skip_group_check=True)

        # software pipeline: keep the PE fed with independent conv work at
        # every L1->L2 boundary (L2(n) waits on ht(n)+halo; L1(n+1) does not)
        phase_l1(0)
        for n in range(BPC):
            if n + 1 < BPC:
                phase_l1(n + 1)
            phase_l2(n)

    nc.compile()
    return nc


def kernel(x, conv1_w, conv2_w, gamma1, beta1, mean1, var1,
           gamma2, beta2, mean2, var2, act_codes_feat, act_codes_sc):
    x = np.asarray(x, np.float32)
    a1 = (np.asarray(gamma1) / np.sqrt(np.asarray(var1) + EPS)).astype(np.float32)
    b1 = (np.asarray(beta1) - np.asarray(mean1) * a1).astype(np.float32)
    a2 = (np.asarray(gamma2) / np.sqrt(np.asarray(var2) + EPS)).astype(np.float32)
    b2 = (np.asarray(beta2) - np.asarray(mean2) * a2).astype(np.float32)

    mf = _mask_arrays(np.asarray(act_codes_feat))
    ms = _mask_arrays(np.asarray(act_codes_sc))

    w1h = np.zeros((9, 128, 128), F16)
    w2h = np.zeros((9, 128, 128), F16)
    for t, (ky, kx) in enumerate(TAPS):
        w1h[t, 0:64, 0:64] = w1h[t, 64:128, 64:128] = \
            np.asarray(conv1_w)[:, :, ky + 1, kx + 1].T.astype(F16)
        w2h[t, 0:64, 0:64] = w2h[t, 64:128, 64:128] = \
            np.asarray(conv2_w)[:, :, ky + 1, kx + 1].T.astype(F16)
    # w1 ships compact: per-partition-half diagonal blocks only
    w1c = np.concatenate([w1h[:, 0:64, 0:64], w1h[:, 64:128, 64:128]], axis=1)
    w1h = np.ascontiguousarray(w1c.transpose(1, 0, 2))  # [128, 9, 64]
    w2h = np.ascontiguousarray(w2h.transpose(1, 0, 2))  # [128, 9, 128]

    dup = lambda v: np.concatenate([v, v]).astype(np.float32)
    sclh = np.stack([dup(a1), dup(b1), dup(a2), dup(b2)], axis=1)  # [128, 4]

    nc = _build_program(fuse_b2_zero=bool(np.all(b2 == 0.0)))

    in_maps = []
    for core in range(NCORES):
        xs = np.stack([
            _pad_split_image(x[core * BPC + i]) for i in range(BPC)
        ])
        in_maps.append({
            "xin": xs,
            "w1": w1h, "w2": w2h, "scl": sclh,
            "s1f": mf["s1"], "ff": mf["f"], "cdf": mf["cd"], "cmf": mf["cm"],
            "s1s": ms["s1"], "fs": ms["f"], "cds": ms["cd"], "cms": ms["cm"],
        })

    res = run_bass_kernel_spmd(nc, in_maps, core_ids=list(range(NCORES)))
    global LAST_RESULT
    LAST_RESULT = res

    out = np.empty((B, C, H, W), np.float32)
    for core in range(NCORES):
        o = res.results[core]["out"]  # [BPC, 128, 56, 112] f16
        for i in range(BPC):
            img = np.concatenate([o[i, 0:64], o[i, 64:128]], axis=1)
            out[core * BPC + i] = img.astype(np.float32)
    return out


if __name__ == "__main__":
    rng = np.random.default_rng(0)
    inputs = {
        "x": rng.standard_normal((B, C, H, W)).astype(np.float32),
        "conv1_w": (rng.standard_normal((C, C, 3, 3)) * 0.05).astype(np.float32),
        "conv2_w": (rng.standard_normal((C, C, 3, 3)) * 0.05).astype(np.float32),
        "gamma1": np.ones(C, np.float32), "beta1": np.zeros(C, np.float32),
        "mean1": np.zeros(C, np.float32), "var1": np.ones(C, np.float32),
        "gamma2": np.ones(C, np.float32), "beta2": np.zeros(C, np.float32),
        "mean2": np.zeros(C, np.float32), "var2": np.ones(C, np.float32),
        "act_codes_feat": rng.integers(0, 4, C * H * W).astype(np.int32),
        "act_codes_sc": rng.integers(0, 4, C * H * W).astype(np.int32),
    }
    out = kernel(**inputs)
    print("out", out.shape, out.dtype, float(np.abs(out).max()))

